# revision 28
# baseline (speedup 1.0000x reference)
"""MultiDirectionalSpatialScanner — Trainium2 Bass kernel, 8 NeuronCores.

Math identities (verified vs reference):
  * scan/restore permutations permute key/value pairs identically within
    each direction; softmax attention is invariant under simultaneous
    permutation of keys+values -> the gather is dropped.
  * Direction projection fuses into K/V projections:
      K_dir = x @ (dir_W[dir] @ wk_h.T) + (dir_b[dir] @ wk_h.T + bk_h)
    The fused weights Weff = dir_W @ [wk.T | wv.T] are precomputed on the
    host, removing the on-device weight-prep phase entirely.
  * out_proj and fin are consecutive linear layers (LayerNorm comes
    after): y = o @ (fin_w @ out_proj_w).T + fin_b_eff -> ONE fused W2
    matmul on device.
  * Scores lie in [-8.8, 8.8] -> unshifted exp is safe; softmax
    normalization deferred past the P@V matmul; the denominator
    sum-over-partitions + broadcast is ONE ones-matmul on TensorE
    (out[m,x] = sum_p ones[p,m]*den[p,x] = replicated column sum).
  * LayerNorm rstd = exp(-0.5*ln(var+eps)) keeps ScalarE on a single
    activation-table set (Exp+Ln share one) -> no table reloads.

Sharding: one attention head per core (H=8). After each batch, the
normalized bf16 oT is exchanged with a per-batch AllToAll (batch's 576
cols = 8 strips of 72; core j gets global rows 576b+72j..+72 from every
head). A2As for batches 0-2 overlap the following batch's compute; each
strip's receiver work (single fused W2 matmul + LayerNorm + residual)
is interleaved into the NEXT batch's instruction stream. Host
reassembles the strip-ordered rows.

All matmuls bf16 (weight loads 2x faster, DMA halves); exp/P@V in bf16
with fp32 PSUM accumulation.
"""

import numpy as np

B, N, D = 4, 576, 1024
K, H, HD = 8, 8, 128
BN = B * N            # 2304
NLOC = BN // 8        # 288
SW = N // 8           # 72, strip width
LN_EPS = 1e-5

_CACHE = {}

ROWCH = [(r, min(128, N - r)) for r in range(0, N, 128)]  # 5 kv-row chunks
NHALF = [(0, 288), (288, 288)]                            # query halves
PSOFF = [0, 512]                                          # PSUM bank offsets


def build():
    import concourse.bacc as bacc
    import concourse.bass as bass
    import concourse.tile as tile
    from concourse import mybir

    F32 = mybir.dt.float32
    BF16 = mybir.dt.bfloat16
    Exp = mybir.ActivationFunctionType.Exp
    Ln = mybir.ActivationFunctionType.Ln

    nc = bacc.Bacc("TRN2", target_bir_lowering=False, debug=False,
                   num_devices=8)

    xT_d = nc.dram_tensor("xT", [D, BN], BF16, kind="ExternalInput").ap()
    wkv_d = nc.dram_tensor("wkv", [D, 2 * D], BF16, kind="ExternalInput").ap()
    wqT_d = nc.dram_tensor("wqT", [D, HD], BF16, kind="ExternalInput").ap()
    w2T_d = nc.dram_tensor("w2T", [D, D], BF16, kind="ExternalInput").ap()
    bq_d = nc.dram_tensor("bq", [HD, 1], F32, kind="ExternalInput").ap()
    bk_d = nc.dram_tensor("bk", [HD, K], F32, kind="ExternalInput").ap()
    bv_d = nc.dram_tensor("bv", [1, D], F32, kind="ExternalInput").ap()
    finb_d = nc.dram_tensor("finb", [1, D], F32, kind="ExternalInput").ap()
    g_d = nc.dram_tensor("g", [1, D], F32, kind="ExternalInput").ap()
    xres_d = nc.dram_tensor("xres", [NLOC, D], F32, kind="ExternalInput").ap()
    out_d = nc.dram_tensor("out", [NLOC, D], F32, kind="ExternalOutput").ap()

    def bcast(ap_1xN, parts):
        a = ap_1xN if isinstance(ap_1xN, bass.AP) else ap_1xN[:]
        return bass.AP(tensor=a.tensor, offset=a.offset,
                       ap=[[0, parts]] + list(a.ap[1:]))

    with tile.TileContext(nc) as tc:
        # per-batch exchange buffers: batch b's 576 cols = 8 strips of 72
        a2a_in, a2a_out, a2a_free = [], [], []
        for b in range(B):
            t_in, f_in = tc.tile([8, 128, SW], BF16, space="DRAM",
                                 name=f"a2a_in{b}")
            t_out, f_out = tc.tile([8, 128, SW], BF16, space="DRAM",
                                   addr_space="Shared", name=f"a2a_out{b}")
            a2a_in.append(t_in)
            a2a_out.append(t_out)
            a2a_free += [f_in, f_out]

        with tc.tile_pool(name="const", bufs=1) as const:
            # startup-critical loads first: wqT (q matmuls gate everything)
            wqT = []
            for c in range(8):
                t = const.tile([128, HD], BF16, tag=f"wqT{c}", name=f"wqT{c}")
                nc.sync.dma_start(out=t, in_=wqT_d[c * 128:(c + 1) * 128, :])
                wqT.append(t)
            bq = const.tile([HD, 1], F32, tag="bq")
            nc.sync.dma_start(out=bq, in_=bq_d)
            bk = const.tile([HD, K], F32, tag="bk")
            nc.sync.dma_start(out=bk, in_=bk_d)
            bv_rep = const.tile([128, D], F32, tag="bv_rep")
            ones = const.tile([128, 128], F32, tag="ones")
            nc.vector.memset(ones, 1.0)
            eps_t = const.tile([128, 1], F32, tag="eps")
            nc.vector.memset(eps_t, LN_EPS)
            # pre-warm the Exp+Ln activation-table set during startup DMA
            # so the first strip-LN doesn't pay a table load mid-batch
            warm = const.tile([1, 1], F32, tag="warm")
            nc.scalar.activation(out=warm, in_=eps_t[:1], func=Ln)
            # WKV tiles: DMAs issued after batch 0's xb loads (V half first
            # since the V projection runs before the K-dir loop)
            WKV = [const.tile([128, 2 * D], BF16, tag=f"WKV{c}", name=f"WKV{c}")
                   for c in range(8)]

            def load_wkv():
                for c in range(8):
                    nc.sync.dma_start(out=WKV[c][:, D:],
                                      in_=wkv_d[c * 128:(c + 1) * 128, D:])
                nc.sync.dma_start(out=bv_rep, in_=bcast(bv_d, 128))
                for c in range(8):
                    nc.sync.dma_start(out=WKV[c][:, 0:D],
                                      in_=wkv_d[c * 128:(c + 1) * 128, 0:D])

            # end-phase constants: DMAs deferred to batch 1
            w2T = [const.tile([128, D], BF16, tag=f"w2T{c}", name=f"w2T{c}")
                   for c in range(8)]
            finb = const.tile([128, D], F32, tag="finb")
            g_rep = const.tile([128, D], F32, tag="g_rep")
            xr_sb = [const.tile([SW, D], F32, tag=f"xr{b}", name=f"xr{b}")
                     for b in range(B)]

            def load_endphase():
                for c in range(8):
                    nc.sync.dma_start(out=w2T[c],
                                      in_=w2T_d[c * 128:(c + 1) * 128, :])
                nc.sync.dma_start(out=finb, in_=bcast(finb_d, 128))
                nc.sync.dma_start(out=g_rep, in_=bcast(g_d, 128))
                for b in range(B):
                    nc.sync.dma_start(out=xr_sb[b],
                                      in_=xres_d[b * SW:(b + 1) * SW, :])

            with tc.tile_pool(name="xbp", bufs=2) as xbp, \
                 tc.tile_pool(name="att", bufs=2) as att, \
                 tc.tile_pool(name="ppool", bufs=6) as ppool, \
                 tc.tile_pool(name="rcv", bufs=2) as rcv, \
                 tc.tile_pool(name="mm_ps", bufs=3, space="PSUM") as mm_ps, \
                 tc.tile_pool(name="o_ps", bufs=1, space="PSUM") as o_ps:

                def recv_strip(b):
                    """Receiver for strip b: rows 576b+72j..+72 (j = core).
                    y = oT^T @ W2T + finb; LayerNorm; +xres; store."""
                    oTf = rcv.tile([128, 8, SW], BF16, tag="oTf",
                                   name=f"oTf{b}")
                    src = a2a_out[b]
                    nc.sync.dma_start(
                        out=oTf,
                        in_=bass.AP(tensor=src.tensor, offset=src.offset,
                                    ap=[[SW, 128], [128 * SW, 8], [1, SW]]))
                    yps = mm_ps.tile([128, 1024], F32, tag="mm")
                    for dch in range(8):
                        for half in range(2):
                            nc.tensor.matmul(
                                yps[:SW, half * 512:(half + 1) * 512],
                                oTf[:, dch, :],
                                w2T[dch][:, half * 512:(half + 1) * 512],
                                start=(dch == 0), stop=(dch == 7))
                    y = rcv.tile([SW, D], F32, tag="y", name=f"y{b}")
                    for half in range(2):
                        nc.vector.tensor_add(
                            y[:, half * 512:(half + 1) * 512],
                            yps[:SW, half * 512:(half + 1) * 512],
                            finb[:SW, half * 512:(half + 1) * 512])
                    stats = rcv.tile([SW, 2, 6], F32, tag="stats")
                    y2 = y.rearrange("p (s x) -> p s x", s=2)
                    for sg in range(2):
                        nc.vector.bn_stats(out=stats[:, sg, :],
                                           in_=y2[:, sg, :])
                    mv = rcv.tile([SW, 2], F32, tag="mv")
                    nc.vector.bn_aggr(out=mv, in_=stats)
                    # rstd = exp(-0.5*ln(var+eps)): stays on the Exp+Ln
                    # activation table set (no reload vs Sqrt)
                    lnv = rcv.tile([SW, 1], F32, tag="lnv")
                    nc.scalar.activation(out=lnv, in_=mv[:, 1:2],
                                         func=Ln, bias=eps_t[:SW])
                    rstd = rcv.tile([SW, 1], F32, tag="rstd")
                    nc.scalar.activation(out=rstd, in_=lnv,
                                         func=Exp, scale=-0.5)
                    negmu = rcv.tile([SW, 1], F32, tag="negmu")
                    nc.vector.tensor_scalar_mul(negmu, mv[:, 0:1], -1.0)
                    nc.vector.tensor_scalar(
                        out=y, in0=y,
                        scalar1=negmu, scalar2=rstd,
                        op0=mybir.AluOpType.add, op1=mybir.AluOpType.mult)
                    nc.vector.tensor_mul(y, y, g_rep[:SW])
                    nc.vector.tensor_add(y, y, xr_sb[b])
                    nc.sync.dma_start(out=out_d[b * SW:(b + 1) * SW, :],
                                      in_=y)

                for b in range(B):
                    r0 = b * N
                    if b == 1:
                        load_endphase()

                    xb = []
                    for c in range(8):
                        t = xbp.tile([128, N], BF16, tag=f"xb{c}", name=f"xb{b}_{c}")
                        nc.sync.dma_start(
                            out=t, in_=xT_d[c * 128:(c + 1) * 128, r0:r0 + N])
                        xb.append(t)
                    if b == 0:
                        load_wkv()

                    # q^T (scaled, biased): [128, 2, 288] bf16
                    qps = mm_ps.tile([128, 1024], F32, tag="mm")
                    for dch in range(8):
                        for hi, (h0, hw) in enumerate(NHALF):
                            nc.tensor.matmul(
                                qps[:, PSOFF[hi]:PSOFF[hi] + hw],
                                wqT[dch], xb[dch][:, h0:h0 + hw],
                                start=(dch == 0), stop=(dch == 7))
                    qb = att.tile([128, 2, 288], BF16, tag="qb")
                    nc.vector.tensor_scalar_add(
                        qb, qps.rearrange("p (h x) -> p h x", h=2)[:, :, 0:288],
                        bq)

                    # V for all 8 dirs: [128, 5, 1024] bf16 (dir-major cols)
                    Vall = att.tile([128, 5, 1024], BF16, tag="Vall",
                                    name=f"Vall{b}")
                    for ri, (rr, rw) in enumerate(ROWCH):
                        vps = mm_ps.tile([128, 1024], F32, tag="mm")
                        for dch in range(8):
                            for half in range(2):
                                nc.tensor.matmul(
                                    vps[:rw, half * 512:(half + 1) * 512],
                                    xb[dch][:, rr:rr + rw],
                                    WKV[dch][:, D + half * 512:
                                             D + (half + 1) * 512],
                                    start=(dch == 0), stop=(dch == 7))
                        for half in range(2):
                            nc.vector.tensor_add(
                                Vall[:rw, ri, half * 512:(half + 1) * 512],
                                vps[:rw, half * 512:(half + 1) * 512],
                                bv_rep[:rw, half * 512:(half + 1) * 512])

                    den = att.tile([128, 2, 288], F32, tag="den")
                    nc.vector.memset(den, 0.0)
                    oT = o_ps.tile([HD, 1024], F32, tag="oT")
                    first_pv = True

                    for kdir in range(K):
                        if kdir == 6 and b >= 1:
                            recv_strip(b - 1)   # A2A #(b-1) done by now
                            # (~32us of firmware+transfer after trigger)

                        # K^T for (b, kdir): [128, 576] bf16
                        ktp = mm_ps.tile([128, 1024], F32, tag="mm")
                        for dch in range(8):
                            for hi, (h0, hw) in enumerate(NHALF):
                                nc.tensor.matmul(
                                    ktp[:, PSOFF[hi]:PSOFF[hi] + hw],
                                    WKV[dch][:, kdir * HD:(kdir + 1) * HD],
                                    xb[dch][:, h0:h0 + hw],
                                    start=(dch == 0), stop=(dch == 7))
                        kt = att.tile([128, N], BF16, tag="kt")
                        kt3 = kt.rearrange("p (h x) -> p h x", h=2)
                        nc.vector.tensor_scalar_add(
                            kt3,
                            ktp.rearrange("p (h x) -> p h x", h=2)[:, :, 0:288],
                            bk[:, kdir:kdir + 1])

                        for ri, (rr, rw) in enumerate(ROWCH):
                            sp = mm_ps.tile([128, 1024], F32, tag="mm")
                            for hi in range(2):
                                nc.tensor.matmul(
                                    sp[:rw, PSOFF[hi]:PSOFF[hi] + 288],
                                    kt[:, rr:rr + rw],
                                    qb[:, hi, :],
                                    start=True, stop=True)
                            pt = ppool.tile([128, 2, 288], BF16, tag="p")
                            nc.scalar.activation(
                                out=pt[:rw],
                                in_=sp.rearrange("p (h x) -> p h x",
                                                 h=2)[:rw, :, 0:288],
                                func=Exp)
                            nc.vector.tensor_add(den[:rw], den[:rw], pt[:rw])
                            last = (kdir == K - 1 and ri == len(ROWCH) - 1)
                            for hi in range(2):
                                nc.tensor.matmul(
                                    oT[:, PSOFF[hi]:PSOFF[hi] + 288],
                                    Vall[:rw, ri,
                                         kdir * HD:(kdir + 1) * HD],
                                    pt[:rw, hi, :],
                                    start=first_pv, stop=last)
                            first_pv = False

                    # denominator: sum over partitions + replicate in ONE
                    # ones-matmul on TensorE; reciprocal on DVE
                    dps = mm_ps.tile([128, 1024], F32, tag="mm")
                    for hi in range(2):
                        nc.tensor.matmul(
                            dps[:, PSOFF[hi]:PSOFF[hi] + 288],
                            ones, den[:, hi, :], start=True, stop=True)
                    rden = att.tile([128, 2, 288], F32, tag="rden")
                    nc.vector.reciprocal_approx_fast(
                        out=rden,
                        in_=dps.rearrange("p (h x) -> p h x", h=2)[:, :, 0:288])

                    # normalize O^T while evacuating PSUM -> bf16 for A2A
                    oT_sb = att.tile([128, 2, 288], BF16, tag="oT_sb")
                    nc.vector.tensor_mul(
                        oT_sb,
                        oT.rearrange("p (h x) -> p h x", h=2)[:, :, 0:288],
                        rden)

                    # ship this batch's 8 strips of 72 cols; A2A #b gives
                    # core j batch-b rows [576b+72j, 576b+72j+72)
                    dst = a2a_in[b]
                    nc.sync.dma_start(
                        out=bass.AP(tensor=dst.tensor, offset=dst.offset,
                                    ap=[[SW, 128], [128 * SW, 8], [1, SW]]),
                        in_=oT_sb)
                    nc.gpsimd.collective_compute(
                        "AllToAll",
                        mybir.AluOpType.bypass,
                        replica_groups=[list(range(8))],
                        ins=[a2a_in[b].opt()],
                        outs=[a2a_out[b].opt()],
                    )

                recv_strip(B - 1)

        for f in a2a_free:
            f()

    nc.compile()
    return nc


def make_in_maps(inputs):
    import ml_dtypes
    bf16 = ml_dtypes.bfloat16

    x = np.asarray(inputs["vision_features"], dtype=np.float32)
    dW = np.asarray(inputs["dir_W"], dtype=np.float32)
    db = np.asarray(inputs["dir_b"], dtype=np.float32)
    ipw = np.asarray(inputs["in_proj_w"], dtype=np.float32)
    ipb = np.asarray(inputs["in_proj_b"], dtype=np.float32)
    opw = np.asarray(inputs["out_proj_w"], dtype=np.float32)
    opb = np.asarray(inputs["out_proj_b"], dtype=np.float32)
    fw = np.asarray(inputs["fin_w"], dtype=np.float32)
    fb = np.asarray(inputs["fin_b"], dtype=np.float32)
    g = np.asarray(inputs["ln_g"], dtype=np.float32)
    lb = np.asarray(inputs["ln_b"], dtype=np.float32)

    wq, wk, wv = ipw[:D], ipw[D:2 * D], ipw[2 * D:]
    bqf, bkf, bvf = ipb[:D], ipb[D:2 * D], ipb[2 * D:]

    x2d = x.reshape(BN, D)
    xT = np.ascontiguousarray(x2d.T).astype(bf16)

    # fused direction+KV weights on host: big[k] = dir_W[k] @ [wk.T|wv.T]
    # (reference: dirs = x @ dir_W[k]; K = dirs @ wk.T)
    wkv_cat = np.concatenate([wk.T, wv.T], axis=1)          # [D, 2D]
    big = np.stack([dW[k] @ wkv_cat for k in range(K)])     # [K, D, 2D]

    bk_eff = db @ wk.T + bkf          # [K, D]
    bv_eff = db @ wv.T + bvf          # [K, D]
    fin_b_eff = (fb + opb @ fw.T).reshape(1, D)
    # out_proj and fin fold into one matrix: y = o @ (fw@opw).T + fin_b_eff
    w2T = np.ascontiguousarray((fw @ opw).T).astype(bf16)   # [D, D]
    sc = 1.0 / np.sqrt(HD)

    in_maps = []
    for h in range(H):
        sl = slice(h * HD, (h + 1) * HD)
        kp = big[:, :, h * HD:(h + 1) * HD]                 # [K, D, HD]
        vp = big[:, :, D + h * HD:D + (h + 1) * HD]
        wkv_h = np.concatenate(
            [kp.transpose(1, 0, 2).reshape(D, K * HD),
             vp.transpose(1, 0, 2).reshape(D, K * HD)], axis=1)
        in_maps.append({
            "xT": xT,
            "wkv": np.ascontiguousarray(wkv_h).astype(bf16),
            "wqT": np.ascontiguousarray(wq[sl].T * sc).astype(bf16),
            "w2T": w2T,
            "bq": np.ascontiguousarray((bqf[sl] * sc)[:, None]),
            "bk": np.ascontiguousarray(bk_eff[:, sl].T),
            "bv": np.ascontiguousarray(bv_eff[:, sl].reshape(1, D)),
            "finb": fin_b_eff,
            "g": g.reshape(1, D),
            # core h's output rows, strip order: global row 576*b + 72*h + t
            "xres": np.ascontiguousarray(
                x2d.reshape(B, 8, SW, D)[:, h].reshape(NLOC, D) + lb),
        })
    return in_maps


def kernel(**inputs):
    from concourse.bass_utils import run_bass_kernel_spmd

    in_maps = make_in_maps(inputs)
    cores = list(range(8))
    if "nc" not in _CACHE:
        _CACHE["nc"] = build()
    res = run_bass_kernel_spmd(_CACHE["nc"], in_maps, cores)
    _CACHE["last_res"] = res
    # core j's out rows are strip-ordered: row b*72+t -> global 576b+72j+t
    stk = np.stack([res.results[h]["out"] for h in range(H)])  # [8,288,D]
    out = stk.reshape(8, B, SW, D).transpose(1, 0, 2, 3).reshape(BN, D)
    return np.ascontiguousarray(out.reshape(B, N, D), dtype=np.float32)


# revision 29
# speedup vs baseline: 1.1632x; 1.1632x over previous
"""MultiDirectionalSpatialScanner — Trainium2 Bass kernel, 8 NeuronCores.

Math identities (verified vs reference):
  * scan/restore permutations permute key/value pairs identically within
    each direction; softmax attention is invariant under simultaneous
    permutation of keys+values -> the gather is dropped.
  * Direction projection fuses into K/V projections:
      K_dir = x @ (dir_W[dir] @ wk_h.T) + (dir_b[dir] @ wk_h.T + bk_h)
    The fused weights Weff = dir_W @ [wk.T | wv.T] are precomputed on the
    host, removing the on-device weight-prep phase entirely.
  * out_proj and fin are consecutive linear layers (LayerNorm comes
    after): y = o @ (fin_w @ out_proj_w).T + fin_b_eff -> ONE fused W2
    matmul on device.
  * Scores lie in [-8.8, 8.8] -> unshifted exp is safe; softmax
    normalization deferred past the P@V matmul; the denominator
    sum-over-partitions + broadcast is ONE ones-matmul on TensorE
    (out[m,x] = sum_p ones[p,m]*den[p,x] = replicated column sum).
  * LayerNorm rstd = exp(-0.5*ln(var+eps)) keeps ScalarE on a single
    activation-table set (Exp+Ln share one) -> no table reloads.

Sharding: one attention head per core (H=8). After each batch, the
normalized bf16 oT is exchanged with a per-batch AllToAll (batch's 576
cols = 8 strips of 72; core j gets global rows 576b+72j..+72 from every
head). A2As for batches 0-2 overlap the following batch's compute; each
strip's receiver work (single fused W2 matmul + LayerNorm + residual)
is interleaved into the NEXT batch's instruction stream. Host
reassembles the strip-ordered rows.

All matmuls bf16 (weight loads 2x faster, DMA halves); exp/P@V in bf16
with fp32 PSUM accumulation.
"""

import numpy as np

B, N, D = 4, 576, 1024
K, H, HD = 8, 8, 128
BN = B * N            # 2304
NLOC = BN // 8        # 288
SW = N // 8           # 72, strip width
LN_EPS = 1e-5

_CACHE = {}

ROWCH = [(r, min(128, N - r)) for r in range(0, N, 128)]  # 5 kv-row chunks
NHALF = [(0, 288), (288, 288)]                            # query halves
PSOFF = [0, 512]                                          # PSUM bank offsets


def build():
    import concourse.bacc as bacc
    import concourse.bass as bass
    import concourse.tile as tile
    from concourse import mybir

    F32 = mybir.dt.float32
    BF16 = mybir.dt.bfloat16
    Exp = mybir.ActivationFunctionType.Exp
    Ln = mybir.ActivationFunctionType.Ln

    nc = bacc.Bacc("TRN2", target_bir_lowering=False, debug=False,
                   num_devices=8)

    xT_d = nc.dram_tensor("xT", [D, BN], BF16, kind="ExternalInput").ap()
    wkv_d = nc.dram_tensor("wkv", [D, 2 * D], BF16, kind="ExternalInput").ap()
    wqT_d = nc.dram_tensor("wqT", [D, HD], BF16, kind="ExternalInput").ap()
    w2T_d = nc.dram_tensor("w2T", [D, D], BF16, kind="ExternalInput").ap()
    bq_d = nc.dram_tensor("bq", [HD, 1], F32, kind="ExternalInput").ap()
    bk_d = nc.dram_tensor("bk", [HD, K], F32, kind="ExternalInput").ap()
    bv_d = nc.dram_tensor("bv", [1, D], F32, kind="ExternalInput").ap()
    finb_d = nc.dram_tensor("finb", [1, D], F32, kind="ExternalInput").ap()
    g_d = nc.dram_tensor("g", [1, D], F32, kind="ExternalInput").ap()
    xres_d = nc.dram_tensor("xres", [NLOC, D], F32, kind="ExternalInput").ap()
    out_d = nc.dram_tensor("out", [NLOC, D], F32, kind="ExternalOutput").ap()

    def bcast(ap_1xN, parts):
        a = ap_1xN if isinstance(ap_1xN, bass.AP) else ap_1xN[:]
        return bass.AP(tensor=a.tensor, offset=a.offset,
                       ap=[[0, parts]] + list(a.ap[1:]))

    with tile.TileContext(nc) as tc:
        # per-batch exchange buffers: batch b's 576 cols = 8 strips of 72
        a2a_in, a2a_out, a2a_free = [], [], []
        for b in range(B):
            t_in, f_in = tc.tile([8, 128, SW], BF16, space="DRAM",
                                 name=f"a2a_in{b}")
            t_out, f_out = tc.tile([8, 128, SW], BF16, space="DRAM",
                                   addr_space="Shared", name=f"a2a_out{b}")
            a2a_in.append(t_in)
            a2a_out.append(t_out)
            a2a_free += [f_in, f_out]

        with tc.tile_pool(name="const", bufs=1) as const:
            # startup-critical loads first: wqT (q matmuls gate everything)
            wqT = []
            for c in range(8):
                t = const.tile([128, HD], BF16, tag=f"wqT{c}", name=f"wqT{c}")
                nc.sync.dma_start(out=t, in_=wqT_d[c * 128:(c + 1) * 128, :])
                wqT.append(t)
            bq = const.tile([HD, 1], F32, tag="bq")
            nc.sync.dma_start(out=bq, in_=bq_d)
            bk = const.tile([HD, K], F32, tag="bk")
            nc.sync.dma_start(out=bk, in_=bk_d)
            bv_rep = const.tile([128, D], F32, tag="bv_rep")
            ones = const.tile([128, 128], F32, tag="ones")
            nc.vector.memset(ones, 1.0)
            eps_t = const.tile([128, 1], F32, tag="eps")
            nc.vector.memset(eps_t, LN_EPS)
            # WKV tiles: DMAs issued after batch 0's xb loads (V half first
            # since the V projection runs before the K-dir loop)
            WKV = [const.tile([128, 2 * D], BF16, tag=f"WKV{c}", name=f"WKV{c}")
                   for c in range(8)]

            def load_wkv():
                for c in range(8):
                    nc.sync.dma_start(out=WKV[c][:, D:],
                                      in_=wkv_d[c * 128:(c + 1) * 128, D:])
                nc.sync.dma_start(out=bv_rep, in_=bcast(bv_d, 128))
                for c in range(8):
                    nc.sync.dma_start(out=WKV[c][:, 0:D],
                                      in_=wkv_d[c * 128:(c + 1) * 128, 0:D])

            # end-phase constants: DMAs deferred to batch 1
            w2T = [const.tile([128, D], BF16, tag=f"w2T{c}", name=f"w2T{c}")
                   for c in range(8)]
            finb = const.tile([128, D], F32, tag="finb")
            g_rep = const.tile([128, D], F32, tag="g_rep")
            xr_sb = [const.tile([SW, D], F32, tag=f"xr{b}", name=f"xr{b}")
                     for b in range(B)]

            def load_endphase():
                for c in range(8):
                    nc.sync.dma_start(out=w2T[c],
                                      in_=w2T_d[c * 128:(c + 1) * 128, :])
                nc.sync.dma_start(out=finb, in_=bcast(finb_d, 128))
                nc.sync.dma_start(out=g_rep, in_=bcast(g_d, 128))
                for b in range(B):
                    nc.sync.dma_start(out=xr_sb[b],
                                      in_=xres_d[b * SW:(b + 1) * SW, :])

            with tc.tile_pool(name="xbp", bufs=2) as xbp, \
                 tc.tile_pool(name="att", bufs=2) as att, \
                 tc.tile_pool(name="ppool", bufs=6) as ppool, \
                 tc.tile_pool(name="rcv", bufs=2) as rcv, \
                 tc.tile_pool(name="mm_ps", bufs=3, space="PSUM") as mm_ps, \
                 tc.tile_pool(name="o_ps", bufs=1, space="PSUM") as o_ps:

                def recv_strip(b):
                    """Receiver for strip b: rows 576b+72j..+72 (j = core).
                    y = oT^T @ W2T + finb; LayerNorm; +xres; store."""
                    oTf = rcv.tile([128, 8, SW], BF16, tag="oTf",
                                   name=f"oTf{b}")
                    src = a2a_out[b]
                    nc.sync.dma_start(
                        out=oTf,
                        in_=bass.AP(tensor=src.tensor, offset=src.offset,
                                    ap=[[SW, 128], [128 * SW, 8], [1, SW]]))
                    yps = mm_ps.tile([128, 1024], F32, tag="mm")
                    for dch in range(8):
                        for half in range(2):
                            nc.tensor.matmul(
                                yps[:SW, half * 512:(half + 1) * 512],
                                oTf[:, dch, :],
                                w2T[dch][:, half * 512:(half + 1) * 512],
                                start=(dch == 0), stop=(dch == 7))
                    y = rcv.tile([SW, D], F32, tag="y", name=f"y{b}")
                    for half in range(2):
                        nc.vector.tensor_add(
                            y[:, half * 512:(half + 1) * 512],
                            yps[:SW, half * 512:(half + 1) * 512],
                            finb[:SW, half * 512:(half + 1) * 512])
                    stats = rcv.tile([SW, 2, 6], F32, tag="stats")
                    y2 = y.rearrange("p (s x) -> p s x", s=2)
                    for sg in range(2):
                        nc.vector.bn_stats(out=stats[:, sg, :],
                                           in_=y2[:, sg, :])
                    mv = rcv.tile([SW, 2], F32, tag="mv")
                    nc.vector.bn_aggr(out=mv, in_=stats)
                    # rstd = exp(-0.5*ln(var+eps)): stays on the Exp+Ln
                    # activation table set (no reload vs Sqrt)
                    lnv = rcv.tile([SW, 1], F32, tag="lnv")
                    nc.scalar.activation(out=lnv, in_=mv[:, 1:2],
                                         func=Ln, bias=eps_t[:SW])
                    rstd = rcv.tile([SW, 1], F32, tag="rstd")
                    nc.scalar.activation(out=rstd, in_=lnv,
                                         func=Exp, scale=-0.5)
                    negmu = rcv.tile([SW, 1], F32, tag="negmu")
                    nc.vector.tensor_scalar_mul(negmu, mv[:, 0:1], -1.0)
                    nc.vector.tensor_scalar(
                        out=y, in0=y,
                        scalar1=negmu, scalar2=rstd,
                        op0=mybir.AluOpType.add, op1=mybir.AluOpType.mult)
                    nc.vector.tensor_mul(y, y, g_rep[:SW])
                    nc.vector.tensor_add(y, y, xr_sb[b])
                    nc.sync.dma_start(out=out_d[b * SW:(b + 1) * SW, :],
                                      in_=y)

                for b in range(B):
                    r0 = b * N
                    if b == 1:
                        load_endphase()

                    xb = []
                    for c in range(8):
                        t = xbp.tile([128, N], BF16, tag=f"xb{c}", name=f"xb{b}_{c}")
                        nc.sync.dma_start(
                            out=t, in_=xT_d[c * 128:(c + 1) * 128, r0:r0 + N])
                        xb.append(t)
                    if b == 0:
                        load_wkv()

                    # q^T (scaled, biased): [128, 2, 288] bf16
                    qps = mm_ps.tile([128, 1024], F32, tag="mm")
                    for dch in range(8):
                        for hi, (h0, hw) in enumerate(NHALF):
                            nc.tensor.matmul(
                                qps[:, PSOFF[hi]:PSOFF[hi] + hw],
                                wqT[dch], xb[dch][:, h0:h0 + hw],
                                start=(dch == 0), stop=(dch == 7))
                    qb = att.tile([128, 2, 288], BF16, tag="qb")
                    nc.vector.tensor_scalar_add(
                        qb, qps.rearrange("p (h x) -> p h x", h=2)[:, :, 0:288],
                        bq)

                    # V for all 8 dirs: [128, 5, 1024] bf16 (dir-major cols)
                    Vall = att.tile([128, 5, 1024], BF16, tag="Vall",
                                    name=f"Vall{b}")
                    for ri, (rr, rw) in enumerate(ROWCH):
                        vps = mm_ps.tile([128, 1024], F32, tag="mm")
                        for dch in range(8):
                            for half in range(2):
                                nc.tensor.matmul(
                                    vps[:rw, half * 512:(half + 1) * 512],
                                    xb[dch][:, rr:rr + rw],
                                    WKV[dch][:, D + half * 512:
                                             D + (half + 1) * 512],
                                    start=(dch == 0), stop=(dch == 7))
                        for half in range(2):
                            nc.vector.tensor_add(
                                Vall[:rw, ri, half * 512:(half + 1) * 512],
                                vps[:rw, half * 512:(half + 1) * 512],
                                bv_rep[:rw, half * 512:(half + 1) * 512])

                    den = att.tile([128, 2, 288], F32, tag="den")
                    nc.vector.memset(den, 0.0)
                    oT = o_ps.tile([HD, 1024], F32, tag="oT")
                    first_pv = True

                    for kdir in range(K):
                        if kdir == 6 and b >= 1:
                            recv_strip(b - 1)   # A2A #(b-1) done by now
                            # (~32us of firmware+transfer after trigger)

                        # K^T for (b, kdir): [128, 576] bf16
                        ktp = mm_ps.tile([128, 1024], F32, tag="mm")
                        for dch in range(8):
                            for hi, (h0, hw) in enumerate(NHALF):
                                nc.tensor.matmul(
                                    ktp[:, PSOFF[hi]:PSOFF[hi] + hw],
                                    WKV[dch][:, kdir * HD:(kdir + 1) * HD],
                                    xb[dch][:, h0:h0 + hw],
                                    start=(dch == 0), stop=(dch == 7))
                        kt = att.tile([128, N], BF16, tag="kt")
                        kt3 = kt.rearrange("p (h x) -> p h x", h=2)
                        nc.vector.tensor_scalar_add(
                            kt3,
                            ktp.rearrange("p (h x) -> p h x", h=2)[:, :, 0:288],
                            bk[:, kdir:kdir + 1])

                        for ri, (rr, rw) in enumerate(ROWCH):
                            sp = mm_ps.tile([128, 1024], F32, tag="mm")
                            for hi in range(2):
                                nc.tensor.matmul(
                                    sp[:rw, PSOFF[hi]:PSOFF[hi] + 288],
                                    kt[:, rr:rr + rw],
                                    qb[:, hi, :],
                                    start=True, stop=True)
                            pt = ppool.tile([128, 2, 288], BF16, tag="p")
                            nc.scalar.activation(
                                out=pt[:rw],
                                in_=sp.rearrange("p (h x) -> p h x",
                                                 h=2)[:rw, :, 0:288],
                                func=Exp)
                            nc.vector.tensor_add(den[:rw], den[:rw], pt[:rw])
                            last = (kdir == K - 1 and ri == len(ROWCH) - 1)
                            for hi in range(2):
                                nc.tensor.matmul(
                                    oT[:, PSOFF[hi]:PSOFF[hi] + 288],
                                    Vall[:rw, ri,
                                         kdir * HD:(kdir + 1) * HD],
                                    pt[:rw, hi, :],
                                    start=first_pv, stop=last)
                            first_pv = False

                    # denominator: sum over partitions + replicate in ONE
                    # ones-matmul on TensorE; reciprocal on DVE
                    dps = mm_ps.tile([128, 1024], F32, tag="mm")
                    for hi in range(2):
                        nc.tensor.matmul(
                            dps[:, PSOFF[hi]:PSOFF[hi] + 288],
                            ones, den[:, hi, :], start=True, stop=True)
                    rden = att.tile([128, 2, 288], F32, tag="rden")
                    nc.vector.reciprocal_approx_fast(
                        out=rden,
                        in_=dps.rearrange("p (h x) -> p h x", h=2)[:, :, 0:288])

                    # normalize O^T while evacuating PSUM -> bf16 for A2A
                    oT_sb = att.tile([128, 2, 288], BF16, tag="oT_sb")
                    nc.vector.tensor_mul(
                        oT_sb,
                        oT.rearrange("p (h x) -> p h x", h=2)[:, :, 0:288],
                        rden)

                    # ship this batch's 8 strips of 72 cols; A2A #b gives
                    # core j batch-b rows [576b+72j, 576b+72j+72)
                    dst = a2a_in[b]
                    nc.sync.dma_start(
                        out=bass.AP(tensor=dst.tensor, offset=dst.offset,
                                    ap=[[SW, 128], [128 * SW, 8], [1, SW]]),
                        in_=oT_sb)
                    nc.gpsimd.collective_compute(
                        "AllToAll",
                        mybir.AluOpType.bypass,
                        replica_groups=[list(range(8))],
                        ins=[a2a_in[b].opt()],
                        outs=[a2a_out[b].opt()],
                    )

                recv_strip(B - 1)

        for f in a2a_free:
            f()

    nc.compile()
    return nc


def make_in_maps(inputs):
    import ml_dtypes
    bf16 = ml_dtypes.bfloat16

    x = np.asarray(inputs["vision_features"], dtype=np.float32)
    dW = np.asarray(inputs["dir_W"], dtype=np.float32)
    db = np.asarray(inputs["dir_b"], dtype=np.float32)
    ipw = np.asarray(inputs["in_proj_w"], dtype=np.float32)
    ipb = np.asarray(inputs["in_proj_b"], dtype=np.float32)
    opw = np.asarray(inputs["out_proj_w"], dtype=np.float32)
    opb = np.asarray(inputs["out_proj_b"], dtype=np.float32)
    fw = np.asarray(inputs["fin_w"], dtype=np.float32)
    fb = np.asarray(inputs["fin_b"], dtype=np.float32)
    g = np.asarray(inputs["ln_g"], dtype=np.float32)
    lb = np.asarray(inputs["ln_b"], dtype=np.float32)

    wq, wk, wv = ipw[:D], ipw[D:2 * D], ipw[2 * D:]
    bqf, bkf, bvf = ipb[:D], ipb[D:2 * D], ipb[2 * D:]

    x2d = x.reshape(BN, D)
    xT = np.ascontiguousarray(x2d.T).astype(bf16)

    # fused direction+KV weights on host: big[k] = dir_W[k] @ [wk.T|wv.T]
    # (reference: dirs = x @ dir_W[k]; K = dirs @ wk.T)
    wkv_cat = np.concatenate([wk.T, wv.T], axis=1)          # [D, 2D]
    big = np.stack([dW[k] @ wkv_cat for k in range(K)])     # [K, D, 2D]

    bk_eff = db @ wk.T + bkf          # [K, D]
    bv_eff = db @ wv.T + bvf          # [K, D]
    fin_b_eff = (fb + opb @ fw.T).reshape(1, D)
    # out_proj and fin fold into one matrix: y = o @ (fw@opw).T + fin_b_eff
    w2T = np.ascontiguousarray((fw @ opw).T).astype(bf16)   # [D, D]
    sc = 1.0 / np.sqrt(HD)

    in_maps = []
    for h in range(H):
        sl = slice(h * HD, (h + 1) * HD)
        kp = big[:, :, h * HD:(h + 1) * HD]                 # [K, D, HD]
        vp = big[:, :, D + h * HD:D + (h + 1) * HD]
        wkv_h = np.concatenate(
            [kp.transpose(1, 0, 2).reshape(D, K * HD),
             vp.transpose(1, 0, 2).reshape(D, K * HD)], axis=1)
        in_maps.append({
            "xT": xT,
            "wkv": np.ascontiguousarray(wkv_h).astype(bf16),
            "wqT": np.ascontiguousarray(wq[sl].T * sc).astype(bf16),
            "w2T": w2T,
            "bq": np.ascontiguousarray((bqf[sl] * sc)[:, None]),
            "bk": np.ascontiguousarray(bk_eff[:, sl].T),
            "bv": np.ascontiguousarray(bv_eff[:, sl].reshape(1, D)),
            "finb": fin_b_eff,
            "g": g.reshape(1, D),
            # core h's output rows, strip order: global row 576*b + 72*h + t
            "xres": np.ascontiguousarray(
                x2d.reshape(B, 8, SW, D)[:, h].reshape(NLOC, D) + lb),
        })
    return in_maps


def kernel(**inputs):
    from concourse.bass_utils import run_bass_kernel_spmd

    in_maps = make_in_maps(inputs)
    cores = list(range(8))
    if "nc" not in _CACHE:
        _CACHE["nc"] = build()
    res = run_bass_kernel_spmd(_CACHE["nc"], in_maps, cores)
    _CACHE["last_res"] = res
    # core j's out rows are strip-ordered: row b*72+t -> global 576b+72j+t
    stk = np.stack([res.results[h]["out"] for h in range(H)])  # [8,288,D]
    out = stk.reshape(8, B, SW, D).transpose(1, 0, 2, 3).reshape(BN, D)
    return np.ascontiguousarray(out.reshape(B, N, D), dtype=np.float32)


# revision 34
# speedup vs baseline: 1.1746x; 1.0098x over previous
"""MultiDirectionalSpatialScanner — Trainium2 Bass kernel, 8 NeuronCores.

Math identities (verified vs reference):
  * scan/restore permutations permute key/value pairs identically within
    each direction; softmax attention is invariant under simultaneous
    permutation of keys+values -> the gather is dropped.
  * Direction projection fuses into K/V projections:
      K_dir = x @ (dir_W[dir] @ wk_h.T) + (dir_b[dir] @ wk_h.T + bk_h)
    The fused weights Weff = dir_W @ [wk.T | wv.T] are precomputed on the
    host, removing the on-device weight-prep phase entirely.
  * out_proj and fin are consecutive linear layers (LayerNorm comes
    after): y = o @ (fin_w @ out_proj_w).T + fin_b_eff -> ONE fused W2
    matmul on device.
  * Scores lie in [-8.8, 8.8] -> unshifted exp is safe; softmax
    normalization deferred past the P@V matmul; the denominator
    sum-over-partitions + broadcast is ONE ones-matmul on TensorE
    (out[m,x] = sum_p ones[p,m]*den[p,x] = replicated column sum).
  * LayerNorm rstd = exp(-0.5*ln(var+eps)) keeps ScalarE on a single
    activation-table set (Exp+Ln share one) -> no table reloads.

Sharding: one attention head per core (H=8). After each batch, the
normalized bf16 oT is exchanged with a per-batch AllToAll (batch's 576
cols = 8 strips of 72; core j gets global rows 576b+72j..+72 from every
head). A2As for batches 0-2 overlap the following batch's compute; each
strip's receiver work (single fused W2 matmul + LayerNorm + residual)
is interleaved into the NEXT batch's instruction stream. Host
reassembles the strip-ordered rows.

All matmuls bf16 (weight loads 2x faster, DMA halves); exp/P@V in bf16
with fp32 PSUM accumulation.
"""

import numpy as np

B, N, D = 4, 576, 1024
K, H, HD = 8, 8, 128
BN = B * N            # 2304
NLOC = BN // 8        # 288
SW = N // 8           # 72, strip width
LN_EPS = 1e-5

_CACHE = {}

ROWCH = [(r, min(128, N - r)) for r in range(0, N, 128)]  # 5 kv-row chunks
NHALF = [(0, 288), (288, 288)]                            # query halves
PSOFF = [0, 512]                                          # PSUM bank offsets


def build():
    import concourse.bacc as bacc
    import concourse.bass as bass
    import concourse.tile as tile
    from concourse import mybir

    F32 = mybir.dt.float32
    BF16 = mybir.dt.bfloat16
    Exp = mybir.ActivationFunctionType.Exp
    Ln = mybir.ActivationFunctionType.Ln

    nc = bacc.Bacc("TRN2", target_bir_lowering=False, debug=False,
                   num_devices=8)

    xT_d = nc.dram_tensor("xT", [D, BN], BF16, kind="ExternalInput").ap()
    wkv_d = nc.dram_tensor("wkv", [D, 2 * D], BF16, kind="ExternalInput").ap()
    wqT_d = nc.dram_tensor("wqT", [D, HD], BF16, kind="ExternalInput").ap()
    w2T_d = nc.dram_tensor("w2T", [D, D], BF16, kind="ExternalInput").ap()
    bq_d = nc.dram_tensor("bq", [HD, 1], F32, kind="ExternalInput").ap()
    bk_d = nc.dram_tensor("bk", [HD, K], F32, kind="ExternalInput").ap()
    bv_d = nc.dram_tensor("bv", [1, D], F32, kind="ExternalInput").ap()
    finb_d = nc.dram_tensor("finb", [1, D], F32, kind="ExternalInput").ap()
    g_d = nc.dram_tensor("g", [1, D], F32, kind="ExternalInput").ap()
    xres_d = nc.dram_tensor("xres", [NLOC, D], F32, kind="ExternalInput").ap()
    out_d = nc.dram_tensor("out", [NLOC, D], F32, kind="ExternalOutput").ap()

    def bcast(ap_1xN, parts):
        a = ap_1xN if isinstance(ap_1xN, bass.AP) else ap_1xN[:]
        return bass.AP(tensor=a.tensor, offset=a.offset,
                       ap=[[0, parts]] + list(a.ap[1:]))

    with tile.TileContext(nc) as tc:
        # per-batch exchange buffers: batch b's 576 cols = 8 strips of 72
        a2a_in, a2a_out, a2a_free = [], [], []
        for b in range(B):
            t_in, f_in = tc.tile([8, 128, SW], BF16, space="DRAM",
                                 name=f"a2a_in{b}")
            t_out, f_out = tc.tile([8, 128, SW], BF16, space="DRAM",
                                   addr_space="Shared", name=f"a2a_out{b}")
            a2a_in.append(t_in)
            a2a_out.append(t_out)
            a2a_free += [f_in, f_out]

        with tc.tile_pool(name="const", bufs=1) as const:
            # startup-critical loads first: wqT (q matmuls gate everything)
            wqT = []
            for c in range(8):
                t = const.tile([128, HD], BF16, tag=f"wqT{c}", name=f"wqT{c}")
                nc.sync.dma_start(out=t, in_=wqT_d[c * 128:(c + 1) * 128, :])
                wqT.append(t)
            bq = const.tile([HD, 1], F32, tag="bq")
            nc.sync.dma_start(out=bq, in_=bq_d)
            bk = const.tile([HD, K], F32, tag="bk")
            nc.sync.dma_start(out=bk, in_=bk_d)
            bv_rep = const.tile([128, D], F32, tag="bv_rep")
            ones = const.tile([128, 128], F32, tag="ones")
            nc.vector.memset(ones, 1.0)
            eps_t = const.tile([128, 1], F32, tag="eps")
            nc.vector.memset(eps_t, LN_EPS)
            # WKV tiles: DMAs issued after batch 0's xb loads (V half first
            # since the V projection runs before the K-dir loop)
            WKV = [const.tile([128, 2 * D], BF16, tag=f"WKV{c}", name=f"WKV{c}")
                   for c in range(8)]

            def load_wkv():
                for c in range(8):
                    nc.sync.dma_start(out=WKV[c][:, D:],
                                      in_=wkv_d[c * 128:(c + 1) * 128, D:])
                nc.sync.dma_start(out=bv_rep, in_=bcast(bv_d, 128))
                for c in range(8):
                    nc.sync.dma_start(out=WKV[c][:, 0:D],
                                      in_=wkv_d[c * 128:(c + 1) * 128, 0:D])

            # end-phase constants: DMAs deferred to batch 1
            w2T = [const.tile([128, D], BF16, tag=f"w2T{c}", name=f"w2T{c}")
                   for c in range(8)]
            finb = const.tile([128, D], F32, tag="finb")
            g_rep = const.tile([128, D], F32, tag="g_rep")
            xr_sb = [const.tile([SW, D], F32, tag=f"xr{b}", name=f"xr{b}")
                     for b in range(B)]

            def load_endphase():
                for c in range(8):
                    nc.sync.dma_start(out=w2T[c],
                                      in_=w2T_d[c * 128:(c + 1) * 128, :])
                nc.sync.dma_start(out=finb, in_=bcast(finb_d, 128))
                nc.sync.dma_start(out=g_rep, in_=bcast(g_d, 128))
                for b in range(B):
                    nc.sync.dma_start(out=xr_sb[b],
                                      in_=xres_d[b * SW:(b + 1) * SW, :])

            with tc.tile_pool(name="xbp", bufs=2) as xbp, \
                 tc.tile_pool(name="att", bufs=2) as att, \
                 tc.tile_pool(name="ppool", bufs=6) as ppool, \
                 tc.tile_pool(name="rcv", bufs=2) as rcv, \
                 tc.tile_pool(name="mm_ps", bufs=3, space="PSUM") as mm_ps, \
                 tc.tile_pool(name="o_ps", bufs=1, space="PSUM") as o_ps:

                def recv_strip(b):
                    """Receiver for strip b: rows 576b+72j..+72 (j = core).
                    y = oT^T @ W2T + finb; LayerNorm; +xres; store."""
                    oTf = rcv.tile([128, 8, SW], BF16, tag="oTf",
                                   name=f"oTf{b}")
                    src = a2a_out[b]
                    nc.sync.dma_start(
                        out=oTf,
                        in_=bass.AP(tensor=src.tensor, offset=src.offset,
                                    ap=[[SW, 128], [128 * SW, 8], [1, SW]]))
                    yps = mm_ps.tile([128, 1024], F32, tag="mm")
                    for dch in range(8):
                        for half in range(2):
                            nc.tensor.matmul(
                                yps[:SW, half * 512:(half + 1) * 512],
                                oTf[:, dch, :],
                                w2T[dch][:, half * 512:(half + 1) * 512],
                                start=(dch == 0), stop=(dch == 7))
                    y = rcv.tile([SW, D], F32, tag="y", name=f"y{b}")
                    for half in range(2):
                        nc.vector.tensor_add(
                            y[:, half * 512:(half + 1) * 512],
                            yps[:SW, half * 512:(half + 1) * 512],
                            finb[:SW, half * 512:(half + 1) * 512])
                    stats = rcv.tile([SW, 2, 6], F32, tag="stats")
                    y2 = y.rearrange("p (s x) -> p s x", s=2)
                    for sg in range(2):
                        nc.vector.bn_stats(out=stats[:, sg, :],
                                           in_=y2[:, sg, :])
                    mv = rcv.tile([SW, 2], F32, tag="mv")
                    nc.vector.bn_aggr(out=mv, in_=stats)
                    # rstd = exp(-0.5*ln(var+eps)): stays on the Exp+Ln
                    # activation table set (no reload vs Sqrt)
                    lnv = rcv.tile([SW, 1], F32, tag="lnv")
                    nc.scalar.activation(out=lnv, in_=mv[:, 1:2],
                                         func=Ln, bias=eps_t[:SW])
                    rstd = rcv.tile([SW, 1], F32, tag="rstd")
                    nc.scalar.activation(out=rstd, in_=lnv,
                                         func=Exp, scale=-0.5)
                    negmu = rcv.tile([SW, 1], F32, tag="negmu")
                    nc.vector.tensor_scalar_mul(negmu, mv[:, 0:1], -1.0)
                    nc.vector.tensor_scalar(
                        out=y, in0=y,
                        scalar1=negmu, scalar2=rstd,
                        op0=mybir.AluOpType.add, op1=mybir.AluOpType.mult)
                    nc.vector.tensor_mul(y, y, g_rep[:SW])
                    nc.vector.tensor_add(y, y, xr_sb[b])
                    nc.sync.dma_start(out=out_d[b * SW:(b + 1) * SW, :],
                                      in_=y)

                def xb_load(b):
                    r0 = b * N
                    xb = []
                    for c in range(8):
                        t = xbp.tile([128, N], BF16, tag=f"xb{c}",
                                     name=f"xb{b}_{c}")
                        nc.sync.dma_start(
                            out=t, in_=xT_d[c * 128:(c + 1) * 128, r0:r0 + N])
                        xb.append(t)
                    if b == 0:
                        load_wkv()
                    if b == 1:
                        load_endphase()
                    return xb

                def qv_compute(b, xb, out):
                    """Generator: q^T + V projections for batch b in 6
                    pieces so they can interleave into the previous
                    batch's exp-paced dir-7 tail."""
                    qps = mm_ps.tile([128, 1024], F32, tag="mm")
                    for dch in range(8):
                        for hi, (h0, hw) in enumerate(NHALF):
                            nc.tensor.matmul(
                                qps[:, PSOFF[hi]:PSOFF[hi] + hw],
                                wqT[dch], xb[dch][:, h0:h0 + hw],
                                start=(dch == 0), stop=(dch == 7))
                    qb = att.tile([128, 2, 288], BF16, tag="qb")
                    nc.vector.tensor_scalar_add(
                        qb, qps.rearrange("p (h x) -> p h x", h=2)[:, :, 0:288],
                        bq)
                    Vall = att.tile([128, 5, 1024], BF16, tag="Vall",
                                    name=f"Vall{b}")
                    out["qb"] = qb
                    out["Vall"] = Vall
                    yield
                    for ri, (rr, rw) in enumerate(ROWCH):
                        vps = mm_ps.tile([128, 1024], F32, tag="mm")
                        for dch in range(8):
                            for half in range(2):
                                nc.tensor.matmul(
                                    vps[:rw, half * 512:(half + 1) * 512],
                                    xb[dch][:, rr:rr + rw],
                                    WKV[dch][:, D + half * 512:
                                             D + (half + 1) * 512],
                                    start=(dch == 0), stop=(dch == 7))
                        for half in range(2):
                            nc.vector.tensor_add(
                                Vall[:rw, ri, half * 512:(half + 1) * 512],
                                vps[:rw, half * 512:(half + 1) * 512],
                                bv_rep[:rw, half * 512:(half + 1) * 512])
                        yield

                cur = {"xb": xb_load(0)}
                for _ in qv_compute(0, cur["xb"], cur):
                    pass

                for b in range(B):
                    qb, Vall, xb = cur["qb"], cur["Vall"], cur["xb"]
                    den = att.tile([128, 2, 288], F32, tag="den")
                    nc.vector.memset(den, 0.0)
                    oT = o_ps.tile([HD, 1024], F32, tag="oT")
                    first_pv = True
                    nxt = {}
                    g = None

                    for kdir in range(K):
                        if kdir == 5 and b + 1 < B:
                            nxt["xb"] = xb_load(b + 1)
                        if kdir == 6 and b >= 1:
                            recv_strip(b - 1)   # A2A #(b-1) done by now
                            # (~32us of firmware+transfer after trigger)
                        if kdir == 7 and b + 1 < B:
                            g = qv_compute(b + 1, nxt["xb"], nxt)

                        # K^T for (b, kdir): [128, 576] bf16
                        ktp = mm_ps.tile([128, 1024], F32, tag="mm")
                        for dch in range(8):
                            for hi, (h0, hw) in enumerate(NHALF):
                                nc.tensor.matmul(
                                    ktp[:, PSOFF[hi]:PSOFF[hi] + hw],
                                    WKV[dch][:, kdir * HD:(kdir + 1) * HD],
                                    xb[dch][:, h0:h0 + hw],
                                    start=(dch == 0), stop=(dch == 7))
                        kt = att.tile([128, N], BF16, tag="kt")
                        kt3 = kt.rearrange("p (h x) -> p h x", h=2)
                        nc.vector.tensor_scalar_add(
                            kt3,
                            ktp.rearrange("p (h x) -> p h x", h=2)[:, :, 0:288],
                            bk[:, kdir:kdir + 1])

                        for ri, (rr, rw) in enumerate(ROWCH):
                            sp = mm_ps.tile([128, 1024], F32, tag="mm")
                            for hi in range(2):
                                nc.tensor.matmul(
                                    sp[:rw, PSOFF[hi]:PSOFF[hi] + 288],
                                    kt[:, rr:rr + rw],
                                    qb[:, hi, :],
                                    start=True, stop=True)
                            pt = ppool.tile([128, 2, 288], BF16, tag="p")
                            nc.scalar.activation(
                                out=pt[:rw],
                                in_=sp.rearrange("p (h x) -> p h x",
                                                 h=2)[:rw, :, 0:288],
                                func=Exp)
                            nc.vector.tensor_add(den[:rw], den[:rw], pt[:rw])
                            last = (kdir == K - 1 and ri == len(ROWCH) - 1)
                            for hi in range(2):
                                nc.tensor.matmul(
                                    oT[:, PSOFF[hi]:PSOFF[hi] + 288],
                                    Vall[:rw, ri,
                                         kdir * HD:(kdir + 1) * HD],
                                    pt[:rw, hi, :],
                                    start=first_pv, stop=last)
                            first_pv = False
                            if g is not None:
                                # fill the exp-paced dir-7 tail with the
                                # next batch's q/V matmuls
                                next(g, None)
                    if g is not None:
                        for _ in g:
                            pass

                    # denominator: sum over partitions + replicate in ONE
                    # ones-matmul on TensorE; reciprocal on DVE
                    dps = mm_ps.tile([128, 1024], F32, tag="mm")
                    for hi in range(2):
                        nc.tensor.matmul(
                            dps[:, PSOFF[hi]:PSOFF[hi] + 288],
                            ones, den[:, hi, :], start=True, stop=True)
                    rden = att.tile([128, 2, 288], F32, tag="rden")
                    nc.vector.reciprocal_approx_fast(
                        out=rden,
                        in_=dps.rearrange("p (h x) -> p h x", h=2)[:, :, 0:288])

                    # normalize O^T while evacuating PSUM -> bf16 for A2A
                    oT_sb = att.tile([128, 2, 288], BF16, tag="oT_sb")
                    nc.vector.tensor_mul(
                        oT_sb,
                        oT.rearrange("p (h x) -> p h x", h=2)[:, :, 0:288],
                        rden)

                    # ship this batch's 8 strips of 72 cols; A2A #b gives
                    # core j batch-b rows [576b+72j, 576b+72j+72)
                    dst = a2a_in[b]
                    nc.sync.dma_start(
                        out=bass.AP(tensor=dst.tensor, offset=dst.offset,
                                    ap=[[SW, 128], [128 * SW, 8], [1, SW]]),
                        in_=oT_sb)
                    nc.gpsimd.collective_compute(
                        "AllToAll",
                        mybir.AluOpType.bypass,
                        replica_groups=[list(range(8))],
                        ins=[a2a_in[b].opt()],
                        outs=[a2a_out[b].opt()],
                    )
                    cur = nxt

                # keep the PE HAM clock warm through the last A2A's
                # ~27us firmware+transfer wait so the strip-3 receiver
                # runs at full clock (junk matmuls, result unused)
                for w in range(16):
                    wp = mm_ps.tile([128, 1024], F32, tag="mm")
                    for i in range(8):
                        nc.tensor.matmul(
                            wp[:, 0:512], WKV[0][:, 0:128],
                            WKV[0][:, 0:512],
                            start=(i == 0), stop=(i == 7))
                recv_strip(B - 1)

        for f in a2a_free:
            f()

    nc.compile()
    return nc


def make_in_maps(inputs):
    import ml_dtypes
    bf16 = ml_dtypes.bfloat16

    x = np.asarray(inputs["vision_features"], dtype=np.float32)
    dW = np.asarray(inputs["dir_W"], dtype=np.float32)
    db = np.asarray(inputs["dir_b"], dtype=np.float32)
    ipw = np.asarray(inputs["in_proj_w"], dtype=np.float32)
    ipb = np.asarray(inputs["in_proj_b"], dtype=np.float32)
    opw = np.asarray(inputs["out_proj_w"], dtype=np.float32)
    opb = np.asarray(inputs["out_proj_b"], dtype=np.float32)
    fw = np.asarray(inputs["fin_w"], dtype=np.float32)
    fb = np.asarray(inputs["fin_b"], dtype=np.float32)
    g = np.asarray(inputs["ln_g"], dtype=np.float32)
    lb = np.asarray(inputs["ln_b"], dtype=np.float32)

    wq, wk, wv = ipw[:D], ipw[D:2 * D], ipw[2 * D:]
    bqf, bkf, bvf = ipb[:D], ipb[D:2 * D], ipb[2 * D:]

    x2d = x.reshape(BN, D)
    xT = np.ascontiguousarray(x2d.T).astype(bf16)

    # fused direction+KV weights on host: big[k] = dir_W[k] @ [wk.T|wv.T]
    # (reference: dirs = x @ dir_W[k]; K = dirs @ wk.T)
    wkv_cat = np.concatenate([wk.T, wv.T], axis=1)          # [D, 2D]
    big = np.stack([dW[k] @ wkv_cat for k in range(K)])     # [K, D, 2D]

    bk_eff = db @ wk.T + bkf          # [K, D]
    bv_eff = db @ wv.T + bvf          # [K, D]
    fin_b_eff = (fb + opb @ fw.T).reshape(1, D)
    # out_proj and fin fold into one matrix: y = o @ (fw@opw).T + fin_b_eff
    w2T = np.ascontiguousarray((fw @ opw).T).astype(bf16)   # [D, D]
    sc = 1.0 / np.sqrt(HD)

    in_maps = []
    for h in range(H):
        sl = slice(h * HD, (h + 1) * HD)
        kp = big[:, :, h * HD:(h + 1) * HD]                 # [K, D, HD]
        vp = big[:, :, D + h * HD:D + (h + 1) * HD]
        wkv_h = np.concatenate(
            [kp.transpose(1, 0, 2).reshape(D, K * HD),
             vp.transpose(1, 0, 2).reshape(D, K * HD)], axis=1)
        in_maps.append({
            "xT": xT,
            "wkv": np.ascontiguousarray(wkv_h).astype(bf16),
            "wqT": np.ascontiguousarray(wq[sl].T * sc).astype(bf16),
            "w2T": w2T,
            "bq": np.ascontiguousarray((bqf[sl] * sc)[:, None]),
            "bk": np.ascontiguousarray(bk_eff[:, sl].T),
            "bv": np.ascontiguousarray(bv_eff[:, sl].reshape(1, D)),
            "finb": fin_b_eff,
            "g": g.reshape(1, D),
            # core h's output rows, strip order: global row 576*b + 72*h + t
            "xres": np.ascontiguousarray(
                x2d.reshape(B, 8, SW, D)[:, h].reshape(NLOC, D) + lb),
        })
    return in_maps


def kernel(**inputs):
    from concourse.bass_utils import run_bass_kernel_spmd

    in_maps = make_in_maps(inputs)
    cores = list(range(8))
    if "nc" not in _CACHE:
        _CACHE["nc"] = build()
    res = run_bass_kernel_spmd(_CACHE["nc"], in_maps, cores)
    _CACHE["last_res"] = res
    # core j's out rows are strip-ordered: row b*72+t -> global 576b+72j+t
    stk = np.stack([res.results[h]["out"] for h in range(H)])  # [8,288,D]
    out = stk.reshape(8, B, SW, D).transpose(1, 0, 2, 3).reshape(BN, D)
    return np.ascontiguousarray(out.reshape(B, N, D), dtype=np.float32)
